# revision 5
# baseline (speedup 1.0000x reference)
"""CoAttention kernel v4 for 8 Trainium2 NeuronCores.

Problem: S, D: [8, 2048, 1024] f32, one batch per core.
  G = D @ S^T                      [2048, 2048]
  co_D = D + rowsoftmax(G) @ S
  co_S = S + rowsoftmax(G^T) @ D

Structure (v4):
 - prologue: load S (full blocks, sync q) + D blocks 0-3 (half blocks,
   gpsimd q); S^T built by PE transposes interleaved with the DMA-bound
   load stream (v2-style); D^T tiles via DMA-xbar transposes on the
   scalar q; G(0)/G(1) chunk-interleaved as S_T columns land.  A dummy
   matmul warmup burst + keep-warm dummies every S block hold the PE
   HAM clock gate at 2.4 GHz (PE transposes don't count as HAM-busy).
 - main loop iter i: G(i+2) + O_D(i); W^T and D^T tiles come from
   DMA-xbar transposes (~1.3-1.9us each, off the PE).  The xbars are
   paced (few DMAs in flight) so the shared DMA-semaphore pool never
   serializes them against loads -- bulk-issuing 16 xbars during the
   load phase was a 100us regression (v3).
 - colsum finalize (16 PE f32 transposes) folded into iters 14/15;
   phase C emits co_S per half with stores split across two queues.
 - residual adds use the resident f16 S_nat/D_nat (~2e-4 extra rel
   err) so phase C reloads nothing.

Softmax trick (v2): shift-invariance with constant SHIFT; shared
W = exp(G - SHIFT) bf16 serves both directions:
  co_D[l] = D[l] + (W @ S)[l] / rowsum_l(W)
  co_S[m] = S[m] + (W^T @ D)[m] / colsum_m(W)
"""

import numpy as np

P = 128
T = 2048
DH = 1024
LT = T // P     # 16 token blocks per side
KD = DH // P    # 8 contraction blocks
NTILE = 512
NCH = T // NTILE  # 4 chunks of the m axis
HB = DH // 2    # half-block D-load granularity
SHIFT = 100.0

DEFAULTS = dict(
    warm_mms=12,
    keep_warm=2,
    stageS_bufs=2,
    stageD_bufs=2,
    gpsum_bufs=2,
    opsum_bufs=1,
    dtp_bufs=4,
    wtp_bufs=2,
    outp_bufs=2,
)

_CACHE = {}


def _build_nc(**overrides):
    import concourse.mybir as mybir
    import concourse.tile as tile
    from concourse import bacc
    from concourse.masks import make_identity

    p = dict(DEFAULTS)
    p.update(overrides)

    dt = mybir.dt
    f32, f16, bf16 = dt.float32, dt.float16, dt.bfloat16
    AX = mybir.AxisListType.X
    EXP = mybir.ActivationFunctionType.Exp
    MULT = mybir.AluOpType.mult
    ADD = mybir.AluOpType.add

    nc = bacc.Bacc("TRN2", target_bir_lowering=False, debug=False)

    S_ap = nc.dram_tensor("S", [T, DH], f32, kind="ExternalInput").ap()
    D_ap = nc.dram_tensor("D", [T, DH], f32, kind="ExternalInput").ap()
    coD_ap = nc.dram_tensor("co_D", [T, DH], f32, kind="ExternalOutput").ap()
    coS_ap = nc.dram_tensor("co_S", [T, DH], f32, kind="ExternalOutput").ap()

    with tile.TileContext(nc) as tc:
        with (
            tc.tile_pool(name="consts", bufs=1) as consts,
            tc.tile_pool(name="big", bufs=1) as big,
            tc.tile_pool(name="stageS", bufs=p["stageS_bufs"]) as stageS,
            tc.tile_pool(name="stageD", bufs=p["stageD_bufs"]) as stageD,
            tc.tile_pool(name="rspp", bufs=3) as rspp,
            tc.tile_pool(name="small", bufs=4) as small,
            tc.tile_pool(name="outp", bufs=p["outp_bufs"]) as outp,
        ):
            ident_f32 = consts.tile([P, P], f32)
            make_identity(nc, ident_f32[:])
            ident_f16 = consts.tile([P, P], f16)
            make_identity(nc, ident_f16[:])
            nbias = consts.tile([P, 1], f32)
            nc.vector.memset(nbias[:], -SHIFT)
            warm_src = consts.tile([P, NTILE], f16)
            nc.vector.memset(warm_src[:], 0.0)

            S_nat = big.tile([P, LT, DH], f16)     # [m%128, (mblk, d)]
            S_T = big.tile([P, KD, T], f16)        # [d%128, (dblk, m)]
            D_nat = big.tile([P, LT, DH], f16)     # [l%128, (lblk, d)]
            W = big.tile([P, LT, T], bf16)         # [l%128, (lblk, m)]
            S1 = big.tile([P, T], f32)             # partial colsums
            nc.vector.memset(S1[:], 0.0)

            gps_ctx = tc.tile_pool(name="gpsum", bufs=p["gpsum_bufs"], space="PSUM")
            gpsum = gps_ctx.__enter__()
            ops_ctx = tc.tile_pool(name="opsum", bufs=p["opsum_bufs"], space="PSUM")
            opsum = ops_ctx.__enter__()
            dtp_ctx = tc.tile_pool(name="dtp", bufs=p["dtp_bufs"])
            dtp = dtp_ctx.__enter__()
            wtp_ctx = tc.tile_pool(name="wtp", bufs=p["wtp_bufs"])
            wtp = wtp_ctx.__enter__()
            wps_ctx = tc.tile_pool(name="warmps", bufs=1, space="PSUM")
            warmps = wps_ctx.__enter__()
            tps_ctx = tc.tile_pool(name="tps", bufs=2, space="PSUM")
            tps = tps_ctx.__enter__()

            # ---- PE warmup: dense dummy MMs while the first loads land ----
            wps = warmps.tile([P, NTILE], f32)

            def _warm(n):
                for _ in range(n):
                    nc.tensor.matmul(wps[:], warm_src[:, 0:P], warm_src[:],
                                     start=True, stop=True)

            _warm(p["warm_mms"])

            def _load_d_half(idx):
                # idx counts half-blocks: block idx//2, half idx%2
                t_ = stageD.tile([P, HB], f32, tag="ldd", name="std")
                nc.gpsimd.dma_start(
                    t_[:], D_ap[(idx // 2) * P:(idx // 2 + 1) * P,
                                (idx % 2) * HB:(idx % 2 + 1) * HB])
                return t_

            def _conv_d_half(idx, t_):
                nc.gpsimd.tensor_copy(
                    D_nat[:, idx // 2, (idx % 2) * HB:(idx % 2 + 1) * HB], t_[:])

            def _mk_dt(i):
                dti = dtp.tile([P, KD, P], f16, tag="dt", name="dt")
                nc.scalar.dma_start(dti[:], D_nat[:, i, :], transpose=True)
                return dti

            def _g_chunk(i, mc, dt_i, rsp):
                gp = gpsum.tile([P, NTILE], f32, tag="g")
                for k in range(KD):
                    nc.tensor.matmul(
                        gp[:],
                        dt_i[:, k, :],
                        S_T[:, k, mc * NTILE:(mc + 1) * NTILE],
                        start=(k == 0),
                        stop=(k == KD - 1),
                    )
                nc.scalar.activation(
                    W[:, i, mc * NTILE:(mc + 1) * NTILE], gp[:], EXP,
                    bias=nbias[:], scale=1.0,
                    accum_out=rsp[:, mc:mc + 1],
                )
                nc.vector.tensor_add(
                    S1[:, mc * NTILE:(mc + 1) * NTILE],
                    S1[:, mc * NTILE:(mc + 1) * NTILE],
                    W[:, i, mc * NTILE:(mc + 1) * NTILE],
                )

            # ---- Prologue ----
            dts = {}
            std_tiles = {}
            rsps = {0: rspp.tile([P, NCH], f32, tag="rsp", name="rsp0"),
                    1: rspp.tile([P, NCH], f32, tag="rsp", name="rsp1")}
            # first D half-loads (blocks 0-1)
            for idx in range(2):
                std_tiles[idx] = _load_d_half(idx)
            # first S loads
            st_tiles = {}
            for j in range(p["stageS_bufs"]):
                st_tiles[j] = stageS.tile([P, DH], f32, tag="ld", name="st")
                nc.sync.dma_start(st_tiles[j][:], S_ap[j * P:(j + 1) * P, :])

            for j in range(LT):
                if j + p["stageS_bufs"] < LT:
                    k = j + p["stageS_bufs"]
                    st_tiles[k] = stageS.tile([P, DH], f32, tag="ld", name="st")
                    nc.sync.dma_start(st_tiles[k][:], S_ap[k * P:(k + 1) * P, :])
                stj = st_tiles.pop(j)
                nc.vector.tensor_copy(S_nat[:, j, :], stj[:])
                # S^T for block j via PE transposes (overlaps DMA-bound loads)
                for g in range(2):
                    pt = tps.tile([P, 4, P], f16, tag="tp")
                    for k4 in range(4):
                        k = g * 4 + k4
                        nc.tensor.transpose(
                            pt[:, k4, :], S_nat[:, j, k * P:(k + 1) * P],
                            ident_f16[:],
                        )
                    nc.vector.tensor_copy(
                        S_T[:, g * 4:(g + 1) * 4, j * P:(j + 1) * P], pt[:]
                    )
                _warm(p["keep_warm"])

                # D pipeline + early G chunks at milestones
                if j == 1:
                    for idx in range(2, 4):
                        _conv_d_half(idx - 2, std_tiles.pop(idx - 2))
                        std_tiles[idx] = _load_d_half(idx)
                elif j == 2:
                    _conv_d_half(2, std_tiles.pop(2))
                    _conv_d_half(3, std_tiles.pop(3))
                    dts[0] = _mk_dt(0)
                    dts[1] = _mk_dt(1)
                elif j == 4:
                    _g_chunk(0, 0, dts[0], rsps[0])
                    _g_chunk(1, 0, dts[1], rsps[1])
                elif j == 5:
                    for idx in range(4, 6):
                        std_tiles[idx] = _load_d_half(idx)
                elif j == 6:
                    _conv_d_half(4, std_tiles.pop(4))
                    _conv_d_half(5, std_tiles.pop(5))
                    std_tiles[6] = _load_d_half(6)
                    std_tiles[7] = _load_d_half(7)
                elif j == 7:
                    _conv_d_half(6, std_tiles.pop(6))
                    _conv_d_half(7, std_tiles.pop(7))
                elif j == 8:
                    _g_chunk(0, 1, dts[0], rsps[0])
                    _g_chunk(1, 1, dts[1], rsps[1])
                    dts[2] = _mk_dt(2)
                    dts[3] = _mk_dt(3)
                elif j == 12:
                    _g_chunk(0, 2, dts[0], rsps[0])
                    _g_chunk(1, 2, dts[1], rsps[1])
            _g_chunk(0, 3, dts[0], rsps[0])
            _g_chunk(1, 3, dts[1], rsps[1])
            dts.pop(0)
            dts.pop(1)

            tps_ctx.__exit__(None, None, None)
            wps_ctx.__exit__(None, None, None)

            # ---- Main loop: iter i runs G(i+2) and O_D(i) ----
            tpsC_ctx = tc.tile_pool(name="tpsC", bufs=2, space="PSUM")
            tpsC = tpsC_ctx.__enter__()
            rcs = None
            cs_p = None
            for i in range(LT):
                # D pipeline: convert block i+3, dt-xbar i+3, load block i+4
                if i + 3 < LT and i + 3 >= 4:
                    _conv_d_half(2 * (i + 3), std_tiles.pop(2 * (i + 3)))
                    _conv_d_half(2 * (i + 3) + 1, std_tiles.pop(2 * (i + 3) + 1))
                    dts[i + 3] = _mk_dt(i + 3)
                if i + 4 < LT:
                    std_tiles[2 * (i + 4)] = _load_d_half(2 * (i + 4))
                    std_tiles[2 * (i + 4) + 1] = _load_d_half(2 * (i + 4) + 1)

                # W row i -> W^T tiles via xbar (exp(i) finished at iter i-2)
                wt = wtp.tile([P, LT, P], bf16, tag="wt")
                nc.scalar.dma_start(wt[:], W[:, i, :], transpose=True)

                if i + 2 < LT:
                    rsps[i + 2] = rspp.tile([P, NCH], f32, tag="rsp",
                                            name="rspn")
                    dt_i = dts.pop(i + 2)
                    for mc in range(NCH):
                        _g_chunk(i + 2, mc, dt_i, rsps[i + 2])

                # colsum finalize folded into the last two iterations
                if i >= LT - 2:
                    if i == LT - 2:
                        cs_p = small.tile([P, LT], f32, tag="csp")
                    base = (i - (LT - 2)) * 8
                    for jj in range(base, base + 8):
                        ptc = tpsC.tile([P, P], f32, tag="tc")
                        nc.tensor.transpose(
                            ptc[:], S1[:, jj * P:(jj + 1) * P], ident_f32[:]
                        )
                        nc.vector.reduce_sum(cs_p[:, jj:jj + 1], ptc[:], axis=AX)
                    if i == LT - 1:
                        rcs = small.tile([P, LT], f32, tag="rcs")
                        nc.vector.reciprocal(rcs[:], cs_p[:])

                rsp = rsps.pop(i)
                rs = small.tile([P, 1], f32, tag="rs")
                nc.vector.reduce_sum(rs[:], rsp[:], axis=AX)
                rrs = small.tile([P, 1], f32, tag="rrs")
                nc.vector.reciprocal(rrs[:], rs[:])

                ps = opsum.tile([P, DH], f32, tag="od")
                for kb in range(LT):
                    for n in range(DH // NTILE):
                        nc.tensor.matmul(
                            ps[:, n * NTILE:(n + 1) * NTILE],
                            wt[:, kb, :],
                            S_nat[:, kb, n * NTILE:(n + 1) * NTILE],
                            start=(kb == 0),
                            stop=(kb == LT - 1),
                        )
                o = outp.tile([P, DH], f32, tag="o")
                nc.vector.scalar_tensor_tensor(
                    o[:], ps[:], rrs[:], D_nat[:, i, :], MULT, ADD
                )
                nc.sync.dma_start(coD_ap[i * P:(i + 1) * P, :], o[:])

            wtp_ctx.__exit__(None, None, None)
            dtp_ctx.__exit__(None, None, None)
            tpsC_ctx.__exit__(None, None, None)
            ops_ctx.__exit__(None, None, None)
            gps_ctx.__exit__(None, None, None)

            # ---- Phase C: O_S = W.T @ D_nat, emit co_S ----
            opc_ctx = tc.tile_pool(name="opc", bufs=2, space="PSUM")
            opc = opc_ctx.__enter__()
            for j in range(LT):
                ps = opc.tile([P, DH], f32, tag="os")
                o_j = outp.tile([P, DH], f32, tag="o", name="o_j")
                for n in range(2):
                    for lb in range(LT):
                        nc.tensor.matmul(
                            ps[:, n * NTILE:(n + 1) * NTILE],
                            W[:, lb, j * P:(j + 1) * P],
                            D_nat[:, lb, n * NTILE:(n + 1) * NTILE],
                            start=(lb == 0),
                            stop=(lb == LT - 1),
                        )
                    # half n complete: emit it while the other half runs
                    hs = slice(n * NTILE, (n + 1) * NTILE)
                    nc.vector.scalar_tensor_tensor(
                        o_j[:, hs], ps[:, hs], rcs[:, j:j + 1],
                        S_nat[:, j, hs], MULT, ADD,
                    )
                    q = nc.gpsimd if j % 2 == 0 else nc.sync
                    q.dma_start(coS_ap[j * P:(j + 1) * P, hs], o_j[:, hs])
            opc_ctx.__exit__(None, None, None)

    nc.compile()
    return nc


def _get_nc():
    if "nc" not in _CACHE:
        import json as _json
        import os as _o
        ov = _json.loads(_o.environ.get("KOPTS", "{}"))
        _CACHE["nc"] = _build_nc(**ov)
    return _CACHE["nc"]


def kernel(S, D):
    from concourse.bass_utils import run_bass_kernel_spmd

    S = np.ascontiguousarray(np.asarray(S, dtype=np.float32))
    D = np.ascontiguousarray(np.asarray(D, dtype=np.float32))
    B = S.shape[0]
    assert S.shape == (B, T, DH) and D.shape == (B, T, DH) and B == 8

    nc = _get_nc()
    in_maps = [{"S": S[b], "D": D[b]} for b in range(B)]
    res = run_bass_kernel_spmd(nc, in_maps, core_ids=list(range(B)))
    co_D = np.stack([res.results[b]["co_D"] for b in range(B)])
    co_S = np.stack([res.results[b]["co_S"] for b in range(B)])
    return (co_D, co_S)
